# revision 10
# baseline (speedup 1.0000x reference)
"""Trainium2 Bass kernel for nn_DirectionalProcessor.

Math: the reference computes, for each pixel p=(h,w):
    out[p] = concat_d( shift_d(x)[p] @ Wd[d] ) @ Wc.T + bc
Because everything is linear, this collapses to an 8-tap 3x3 convolution
(zero center tap) with per-tap fused matrices:
    M_d = Wd[d] @ Wc[:, d*C:(d+1)*C].T          (C x C)
    out[p] = sum_d x[p - (dy_d, dx_d)] @ M_d + bc
This halves the FLOPs vs. the reference formulation. M_d is folded on the
host (weight preprocessing, fp64 accumulate -> fp16) so the device spends
zero PE time or DMA-dependency depth on it.

Sharding: data-parallel over batch. 16 images / 8 cores = 2 images per core.
Weights are replicated to every core. No collectives.

Host does layout + weight fold only:
  - grid  -> fp16 channel-major, zero-padded flat [2, 2, 128, 4358] per core
             (66x66 spatially padded image + 1 sentinel zero at each end,
             so every shifted tap window is a contiguous 1-D slice); plus a
             pre-sliced fp8 copy of the tap-3/7 windows [2, 2, 128, 33, 2, 128]
  - M     -> [p=c%128, d, c_chunk, o] fp16 (exact SBUF layout, line-rate DMA)
  - bias  -> pre-broadcast and pre-scaled by s: [128, 512] fp32 (preloaded
             into the fp8 PSUM bank, so the evacuation is a single DVE op)
Device pipeline per core (v4; v3 was 123.6us, v1 142.8us):
  - THREE DMA issue queues (each DMA_DIRECT2D costs ~0.6-0.7us of queue
    issue time, so queue parallelism is what sets the conv-loop start):
    Sync/HWDGE takes m d0-2,d4 + inv_s + mid x chunks; Scalar/HWDGE (idle
    engine, also HWDGE-capable) takes the x head strips + bias + m d5-6 +
    m8 + x tails; the SWDGE ring takes x8 strips and all of image 1.
  - PE ramp: 2 free-running warmup matmuls + 1 gated on each of m d0/d1,
    then the conv loop's own per-chunk DMA gates continue the ramp --
    a dense burst trips the HAM power clamp to half clock (measured on v2).
  - main loop: out tile = 128 consecutive *padded* positions x 256 ch;
    16 accumulating matmuls per tile (8 taps x 2 c-chunks); lhsT is a
    contiguous 128-wide window of the padded channel-major image (the BIR
    verifier requires the stationary operand AP to be 1-D, so pad columns
    are computed as garbage and sliced away on the host).
  - taps d3/d7 run as fp8e4 DoubleRow matmuls (K=256 in one pass at 2x
    rate) into a second PSUM bank preloaded with bias*s by the Scalar
    engine; M_d{3,7} are host-scaled by a power of two into e4m3 range.
    (2 fp8 taps is the max: each adds ~1.3e-2 rel err in quadrature and
    the gate is 2e-2; measured 1.855e-2, same-seed deterministic.)
  - single PSUM pool rotates all 8 banks (warmup bank is reclaimed).
  - evacuation is ONE DVE op per pair: ot = (pt8 * inv_s) + pt, fp16;
    stores alternate between the two HWDGE queues so the final store
    never serializes behind the previous one's issue slot.
  - host casts the fp16 output back to fp32 (adds ~2e-4 rel err; the
    fp16 PE path is already ~4e-4).
"""

import numpy as np

import concourse.bass as bass
import concourse.bacc as bacc
import concourse.mybir as mybir
import concourse.tile as tile
from concourse.bass_utils import run_bass_kernel_spmd

B, H, W, C = 16, 64, 64, 256
DIRECTIONS = [(0, -1), (1, -1), (1, 0), (1, 1), (0, 1), (-1, 1), (-1, 0), (-1, -1)]
N_CORES = 8
BPC = B // N_CORES  # images per core
HP = H + 2  # 66: padded spatial extent
XF = HP * HP + 2  # 4358: flat padded image + sentinel zero at each end
NQ = H * HP  # 4224: padded output positions per image (rows 1..64, all wp)
NT = (NQ + 127) // 128  # 33 output tiles per image
F16 = mybir.dt.float16
F32 = mybir.dt.float32
F8 = mybir.dt.float8e4
FP8_TAPS = [3, 7]  # direction indices computed in fp8 DoubleRow
# both taps have delta = +/-67, so their windows tile the flat buffer at
# stride 128 exactly (offsets 0 and 134) and can be host-pre-sliced into
# contiguous [p, j, ch, 128] DoubleRow lhsT layouts

S0 = 704  # head-strip columns: covers conv tiles 0-3 before chunk 2 lands
S1 = 2624  # second chunk boundary: covers tiles through j=18

LAST_RESULTS = None  # test.py reads this for profiling info


def build_bass() -> bass.Bass:
    nc = bacc.Bacc(None)

    xp_d = nc.dram_tensor("xp", [BPC, 2, 128, XF], F16, kind="ExternalInput")
    x8_d = nc.dram_tensor("x8", [BPC, 2, 128, NT, 2, 128], F8, kind="ExternalInput")
    m_d = nc.dram_tensor("m", [128, 8, 2, C], F16, kind="ExternalInput")
    m8_d = nc.dram_tensor("m8", [128, 2, 2, C], F8, kind="ExternalInput")
    is_d = nc.dram_tensor("inv_s", [128, 1], F32, kind="ExternalInput")
    b_d = nc.dram_tensor("bias", [128, 512], F32, kind="ExternalInput")
    out_d = nc.dram_tensor("out", [BPC * NQ, C], F16, kind="ExternalOutput")

    with tile.TileContext(nc) as tc:
        with (
            tc.tile_pool(name="const", bufs=1) as const,
            tc.tile_pool(name="psum", bufs=8, space="PSUM") as psum_pool,
            tc.tile_pool(name="osb", bufs=6) as osb_pool,
        ):
            m16 = const.tile([128, 8, 2, C], F16, tag="m16")
            m8t = const.tile([128, 2, 2, C], F8, tag="m8")
            bias_sb = const.tile([128, 512], F32, tag="bias_sb")
            inv_s = const.tile([128, 1], F32, tag="inv_s")
            xts = []  # [img][chunk] -> tile [128, XF]
            for img in range(BPC):
                xts.append(
                    [
                        const.tile(
                            [128, XF], F16, tag=f"xp_{img}_{ch}", name=f"xp_{img}_{ch}"
                        )
                        for ch in range(2)
                    ]
                )
            # tap-3/7 windows (delta=-/+67) tile the flat image at stride 128
            # with no overlap, so the host pre-slices them into [p, j, ch, 128]
            # -- each tile's DoubleRow lhsT is then fully contiguous (the ISA
            # dual-fp8 LDWEIGHTS rejects strided k-pairs).
            x8ts = [
                [
                    const.tile(
                        [128, NT, 2, 128], F8, tag=f"x8_{img}_{t}", name=f"x8_{img}_{t}"
                    )
                    for t in range(2)
                ]
                for img in range(BPC)
            ]

            # ---- startup DMAs, three issue queues in parallel ----
            # Sync (HWDGE #1): the m chunks the conv loop consumes first.
            for d in (0, 1, 2, 4):
                nc.sync.dma_start(out=m16[:, d], in_=m_d[:][:, d])
            nc.sync.dma_start(out=inv_s[:], in_=is_d[:])
            # Scalar (HWDGE #2): x head strips first (conv gate), then bias
            # (needed by the PSUM preload ~1.3us after conv start), then the
            # back-half m chunks and m8. d3/d7 fp16 copies are unused.
            for ch in range(2):
                nc.scalar.dma_start(out=xts[0][ch][:, 0:S0], in_=xp_d[:][0, ch, :, 0:S0])
            nc.scalar.dma_start(out=bias_sb[:], in_=b_d[:])
            for d in (5, 6):
                nc.scalar.dma_start(out=m16[:, d], in_=m_d[:][:, d])
            nc.scalar.dma_start(out=m8t[:], in_=m8_d[:])
            # SWDGE ring: x8 strips (first needed ~1.3us after conv start),
            # then the x8 remainders, then all of image 1.
            for t in range(2):
                nc.gpsimd.dma_start(out=x8ts[0][t][:, 0:8], in_=x8_d[:][0, t, :, 0:8])
            for t in range(2):
                nc.gpsimd.dma_start(out=x8ts[0][t][:, 8:NT], in_=x8_d[:][0, t, :, 8:NT])
            # img0 x remainder: mid chunk on Sync, tail on Scalar (issue
            # slots there free up right when these are needed)
            for ch in range(2):
                nc.sync.dma_start(
                    out=xts[0][ch][:, S0:S1], in_=xp_d[:][0, ch, :, S0:S1]
                )
            for ch in range(2):
                nc.scalar.dma_start(
                    out=xts[0][ch][:, S1:XF], in_=xp_d[:][0, ch, :, S1:XF]
                )
            for ch in range(2):  # img1 whole
                nc.gpsimd.dma_start(out=xts[1][ch][:], in_=xp_d[:][1, ch])
            for t in range(2):
                nc.gpsimd.dma_start(out=x8ts[1][t][:], in_=x8_d[:][1, t])

            # ---- PE pre-warm, ramped: 2 free-running matmuls as soon as the
            # engine boots, then 1 per early M-chunk arrival; the conv loop's
            # own DMA gates continue the ramp to 100% duty ----
            warm16 = const.tile([128, 256], F16, tag="warm16")
            nc.vector.memset(warm16[:], 0.0)
            wps = psum_pool.tile([128, 512], F32, tag="ps", name="warm")
            for _ in range(2):
                nc.tensor.matmul(
                    wps[:, 0:256], lhsT=warm16[:, 0:128], rhs=warm16[:]
                )
            for d in (0, 1):
                nc.tensor.matmul(
                    wps[:, 0:256], lhsT=warm16[:, 0:128], rhs=m16[:, d, 0, :]
                )

            # ---- main conv loop ----
            # tile j = padded positions q in [66 + 128j, 66 + 128j + 128);
            # tap d reads the flat buffer at 67 + 128j + delta_d (contiguous).
            # pt accumulates taps d0-d6 (12 fp16 matmuls per half); pt8 is
            # preloaded with bias*s by the Scalar engine and takes taps d3/d7
            # as fp8 DoubleRow matmuls. The DVE cannot read two PSUM inputs
            # in one op (NCC_IBVF027), so the DR matmuls run FIRST in each
            # pair (except pair 0, whose startup gating wants the fp16 taps
            # first) and the Scalar engine rescales pt8 -> t8 (SBUF) while
            # the fp16 taps are still accumulating; the only post-matmul op
            # is then one DVE add + the store.
            deltas = [-(dy * HP + dx) for (dx, dy) in DIRECTIONS]
            pair_idx = 0
            for img in range(BPC):
                x0, x1 = xts[img][0], xts[img][1]
                for jp in range((NT + 1) // 2):
                    pair = [j for j in (2 * jp, 2 * jp + 1) if j < NT]
                    pw = 256 * len(pair)
                    pt = psum_pool.tile([128, 512], F32, tag="ps", name=f"ps{img}_{jp}")
                    pt8 = psum_pool.tile(
                        [128, 512], F32, tag="ps", name=f"ps8{img}_{jp}"
                    )

                    def emit_dr(half, j):
                        for ti in range(2):
                            nc.tensor.matmul(
                                pt8[:, half * 256 : (half + 1) * 256],
                                lhsT=x8ts[img][ti][:, j],
                                rhs=m8t[:, ti],
                                start=(ti == 0),
                                stop=(ti == 1),
                                perf_mode=mybir.MatmulPerfMode.DoubleRow,
                            )

                    def emit_f16(half, j):
                        for di in range(8):
                            if di in FP8_TAPS:
                                continue
                            s = 67 + 128 * j + deltas[di]
                            for ch, xt in enumerate((x0, x1)):
                                nc.tensor.matmul(
                                    pt[:, half * 256 : (half + 1) * 256],
                                    lhsT=xt[:, s : s + 128],
                                    rhs=m16[:, di, ch, :],
                                    start=(di == 0 and ch == 0),
                                    stop=(di == 6 and ch == 1),
                                )

                    if pair_idx == 0:
                        # startup: m8/x8 arrive after the m16 front half
                        for half, j in enumerate(pair):
                            emit_f16(half, j)
                        for half, j in enumerate(pair):
                            emit_dr(half, j)
                    else:
                        for half, j in enumerate(pair):
                            emit_dr(half, j)
                        for half, j in enumerate(pair):
                            emit_f16(half, j)
                    # pt8 rescale + bias add run on the (otherwise idle)
                    # Scalar and GpSimd engines while the fp16 taps are
                    # still accumulating into pt: the only op left on the
                    # post-matmul critical path is one DVE add.
                    t8a = osb_pool.tile([128, 512], F32, tag="t8a", name=f"t8a{img}_{jp}")
                    t8b = osb_pool.tile([128, 512], F32, tag="t8b", name=f"t8b{img}_{jp}")
                    nc.scalar.mul(t8a[:, :pw], pt8[:, :pw], inv_s[:])
                    nc.gpsimd.tensor_add(t8b[:, :pw], t8a[:, :pw], bias_sb[:, :pw])
                    ot = osb_pool.tile([128, 512], F16, tag="osb", name=f"ot{img}_{jp}")
                    nc.vector.tensor_add(ot[:, :pw], pt[:, :pw], t8b[:, :pw])
                    # store: out rows = img*NQ + 128*j + p, contiguous per tile
                    base_row = img * NQ + 128 * pair[0]
                    dst = out_d[:][base_row : base_row + 128 * len(pair), :].rearrange(
                        "(j p) o -> p j o", p=128
                    )
                    nc.sync.dma_start(
                        out=dst,
                        in_=ot[:, :pw].rearrange("p (j o) -> p j o", o=256),
                    )
                    pair_idx += 1

    nc.finalize()  # Bacc: run reg-alloc + sync-wait splitting before serialization
    return nc


def _host_prep(grid_embedding, Wd, Wc, bc):
    g = np.asarray(grid_embedding, dtype=np.float32)
    gpad = np.zeros((B, C, XF), np.float16)
    gview = gpad[:, :, 1 : 1 + HP * HP].reshape(B, C, HP, HP)
    gview[:, :, 1 : H + 1, 1 : W + 1] = g.transpose(0, 3, 1, 2)
    xp = gpad.reshape(B, 2, 128, XF)
    # fold: M[d, c, o] = sum_e Wd[d, c, e] * Wc[o, d*C + e], fp64 accumulate
    wcr = np.asarray(Wc, np.float64).reshape(C, 8, C)  # [o, d, e]
    M = np.einsum("dce,ode->dco", np.asarray(Wd, np.float64), wcr)
    m = np.ascontiguousarray(
        M.reshape(8, 2, 128, C).transpose(2, 0, 1, 3).astype(np.float16)
    )  # [p=c%128, d, c_chunk, o]
    # fp8 tap: global power-of-2 scale into e4m3 range
    import ml_dtypes

    absmax = max(float(np.abs(M[d]).max()) for d in FP8_TAPS)
    s = 2.0 ** np.floor(np.log2(448.0 / max(absmax, 1e-30) / 2.0))
    bias = np.ascontiguousarray(
        np.broadcast_to(np.tile(np.asarray(bc, np.float32), 2), (128, 512))
    )
    m8 = np.ascontiguousarray(
        np.stack(
            [(M[d] * s).reshape(2, 128, C).transpose(1, 0, 2) for d in FP8_TAPS],
            axis=1,
        )
    ).astype(ml_dtypes.float8_e4m3)  # [p=c%128, tap, ch, o]
    x8flat = xp.astype(ml_dtypes.float8_e4m3)  # [img, ch, p, flat]
    slices = []
    for d in FP8_TAPS:
        off = 67 - (DIRECTIONS[d][1] * HP + DIRECTIONS[d][0])
        slices.append(
            x8flat[:, :, :, off : off + NT * 128]
            .reshape(B, 2, 128, NT, 128)
            .transpose(0, 2, 3, 1, 4)
        )  # [img, p, j, ch, 128]
    x8 = np.ascontiguousarray(np.stack(slices, axis=1))  # [img, tap, p, j, ch, w]
    inv_s = np.full((128, 1), 1.0 / s, np.float32)
    return xp, m, bias, x8, m8, inv_s


_NC_CACHE = {}


def _unpad_out(outpad_flat):
    # [NQ*images, 256] f16 -> [images, H, W, C]: rows are (hp-1, wp) for padded
    # rows hp in 1..64 and all wp in 0..65; discard wp 0 and 65.
    n_img = outpad_flat.shape[0] // NQ
    o = outpad_flat.reshape(n_img, H, HP, C)
    return o[:, :, 1 : W + 1, :]


def kernel(grid_embedding, Wd, Wc, bc):
    global LAST_RESULTS
    xp, m, bias, x8, m8, inv_s = _host_prep(grid_embedding, Wd, Wc, bc)

    if "nc" not in _NC_CACHE:
        _NC_CACHE["nc"] = build_bass()
    nc = _NC_CACHE["nc"]

    in_maps = [
        {
            "xp": np.ascontiguousarray(xp[core * BPC : (core + 1) * BPC]),
            "x8": np.ascontiguousarray(x8[core * BPC : (core + 1) * BPC]),
            "m": m,
            "m8": m8,
            "inv_s": inv_s,
            "bias": bias,
        }
        for core in range(N_CORES)
    ]
    res = run_bass_kernel_spmd(nc, in_maps, core_ids=list(range(N_CORES)))
    LAST_RESULTS = res
    out = np.concatenate([_unpad_out(r["out"]) for r in res.results], axis=0)
    return np.ascontiguousarray(out.astype(np.float32))


if __name__ == "__main__":
    rng = np.random.default_rng(0)
    inputs = {
        "grid_embedding": rng.standard_normal((B, H, W, C), dtype=np.float32),
        "Wd": (rng.standard_normal((8, C, C)) * 0.01).astype(np.float32),
        "Wc": (rng.standard_normal((C, 8 * C)) * 0.02).astype(np.float32),
        "bc": (rng.standard_normal(C) * 0.02).astype(np.float32),
    }
    out = kernel(**inputs)
    print("out", out.shape, out.dtype)


# revision 21
# speedup vs baseline: 1.1521x; 1.1521x over previous
"""Trainium2 Bass kernel for nn_DirectionalProcessor.

Math: the reference computes, for each pixel p=(h,w):
    out[p] = concat_d( shift_d(x)[p] @ Wd[d] ) @ Wc.T + bc
Because everything is linear, this collapses to an 8-tap 3x3 convolution
(zero center tap) with per-tap fused matrices:
    M_d = Wd[d] @ Wc[:, d*C:(d+1)*C].T          (C x C)
    out[p] = sum_d x[p - (dy_d, dx_d)] @ M_d + bc
This halves the FLOPs vs. the reference formulation. M_d is folded on the
host (weight preprocessing, fp64 accumulate -> fp16) so the device spends
zero PE time or DMA-dependency depth on it.

Sharding: data-parallel over batch. 16 images / 8 cores = 2 images per core.
Weights are replicated to every core. No collectives.

Host does layout + weight fold only:
  - grid  -> fp16 channel-major, zero-padded flat [2, 2, 128, 4358] per core
             (66x66 spatially padded image + 1 sentinel zero at each end,
             so every shifted tap window is a contiguous 1-D slice); plus a
             pre-sliced fp8 copy of the tap-3/7 windows [2, 2, 128, 33, 2, 128]
  - M     -> [p=c%128, d, c_chunk, o] fp16 (exact SBUF layout, line-rate DMA)
  - bias  -> pre-broadcast and pre-scaled by s: [128, 512] fp32 (preloaded
             into the fp8 PSUM bank, so the evacuation is a single DVE op)
Device pipeline per core (v4; v3 was 123.6us, v1 142.8us):
  - THREE DMA issue queues (each DMA_DIRECT2D costs ~0.6-0.7us of queue
    issue time, so queue parallelism is what sets the conv-loop start):
    Sync/HWDGE takes m d0-2,d4 + inv_s + mid x chunks; Scalar/HWDGE (idle
    engine, also HWDGE-capable) takes the x head strips + bias + m d5-6 +
    m8 + x tails; the SWDGE ring takes x8 strips and all of image 1.
  - PE ramp: 2 free-running warmup matmuls + 1 gated on each of m d0/d1,
    then the conv loop's own per-chunk DMA gates continue the ramp --
    a dense burst trips the HAM power clamp to half clock (measured on v2).
  - main loop: out tile = 128 consecutive *padded* positions x 256 ch;
    16 accumulating matmuls per tile (8 taps x 2 c-chunks); lhsT is a
    contiguous 128-wide window of the padded channel-major image (the BIR
    verifier requires the stationary operand AP to be 1-D, so pad columns
    are computed as garbage and sliced away on the host).
  - taps d3/d7 run as fp8e4 DoubleRow matmuls (K=256 in one pass at 2x
    rate) into a second PSUM bank preloaded with bias*s by the Scalar
    engine; M_d{3,7} are host-scaled by a power of two into e4m3 range.
    (2 fp8 taps is the max: each adds ~1.3e-2 rel err in quadrature and
    the gate is 2e-2; measured 1.855e-2, same-seed deterministic.)
  - single PSUM pool rotates all 8 banks (warmup bank is reclaimed).
  - evacuation is ONE DVE op per pair: ot = (pt8 * inv_s) + pt, fp16;
    stores alternate between the two HWDGE queues so the final store
    never serializes behind the previous one's issue slot.
  - host casts the fp16 output back to fp32 (adds ~2e-4 rel err; the
    fp16 PE path is already ~4e-4).
"""

import numpy as np

import concourse.bass as bass
import concourse.bacc as bacc
import concourse.mybir as mybir
import concourse.tile as tile
from concourse.bass_utils import run_bass_kernel_spmd

B, H, W, C = 16, 64, 64, 256
DIRECTIONS = [(0, -1), (1, -1), (1, 0), (1, 1), (0, 1), (-1, 1), (-1, 0), (-1, -1)]
N_CORES = 8
BPC = B // N_CORES  # images per core
HP = H + 2  # 66: padded spatial extent
XF = HP * HP + 2  # 4358: flat padded image + sentinel zero at each end
NQ = H * HP  # 4224: padded output positions per image (rows 1..64, all wp)
NT = (NQ + 127) // 128  # 33 output tiles per image
F16 = mybir.dt.float16
F32 = mybir.dt.float32
F8 = mybir.dt.float8e4
FP8_TAPS = [3, 7]  # direction indices computed in fp8 DoubleRow
# both taps have delta = +/-67, so their windows tile the flat buffer at
# stride 128 exactly (offsets 0 and 134) and can be host-pre-sliced into
# contiguous [p, j, ch, 128] DoubleRow lhsT layouts

S0 = 512  # head-strip columns: covers conv tiles 0-1 before chunk 2 lands

LAST_RESULTS = None  # test.py reads this for profiling info


def build_bass() -> bass.Bass:
    nc = bacc.Bacc(None)

    xp_d = nc.dram_tensor("xp", [BPC, 2, 128, XF], F16, kind="ExternalInput")
    x8_d = nc.dram_tensor("x8", [BPC, 2, 128, NT, 2, 128], F8, kind="ExternalInput")
    m_d = nc.dram_tensor("m", [128, 8, 2, C], F16, kind="ExternalInput")
    m8_d = nc.dram_tensor("m8", [128, 2, 2, C], F8, kind="ExternalInput")
    # bias pre-broadcast to [128, 512] with 1/s appended as column 512 --
    # folding inv_s into this DMA saves a whole 128-descriptor transfer
    # on the startup-critical ring
    b_d = nc.dram_tensor("bias", [128, 520], F32, kind="ExternalInput")
    out_d = nc.dram_tensor("out", [BPC * NQ, C], F16, kind="ExternalOutput")

    with tile.TileContext(nc) as tc:
        with (
            tc.tile_pool(name="const", bufs=1) as const,
            tc.tile_pool(name="psum", bufs=8, space="PSUM") as psum_pool,
            tc.tile_pool(name="osb", bufs=6) as osb_pool,
        ):
            m16 = const.tile([128, 8, 2, C], F16, tag="m16")
            m8t = const.tile([128, 2, 2, C], F8, tag="m8")
            bias_sb = const.tile([128, 520], F32, tag="bias_sb")
            inv_s = bias_sb[:, 512:513]
            xts = []  # [img][chunk] -> tile [128, XF]
            for img in range(BPC):
                xts.append(
                    [
                        const.tile(
                            [128, XF], F16, tag=f"xp_{img}_{ch}", name=f"xp_{img}_{ch}"
                        )
                        for ch in range(2)
                    ]
                )
            # tap-3/7 windows (delta=-/+67) tile the flat image at stride 128
            # with no overlap, so the host pre-slices them into [p, j, ch, 128]
            # -- each tile's DoubleRow lhsT is then fully contiguous (the ISA
            # dual-fp8 LDWEIGHTS rejects strided k-pairs).
            x8ts = [
                [
                    const.tile(
                        [128, NT, 2, 128], F8, tag=f"x8_{img}_{t}", name=f"x8_{img}_{t}"
                    )
                    for t in range(2)
                ]
                for img in range(BPC)
            ]

            # ---- startup DMAs ----
            # The DMA *transfers* are descriptor-rate-bound (~128 rows per
            # DMA); the SWDGE ring drains descriptors several times faster
            # than the two HWDGE queues, so the startup-critical sequence
            # lives on the ring in need-order, and the HWDGE queues take a
            # few m chunks each in parallel. (Putting the bulk on HWDGE --
            # tried in v4 -- pushed ring completions out to ~30us and
            # starved the conv loop for its first 25us.)
            # Sync (HWDGE #1): front m chunks + m8. The dead d3 fp16 copy
            # is no longer transferred (tap 3 runs in fp8).
            for d in (0, 1, 2):
                nc.sync.dma_start(out=m16[:, d], in_=m_d[:][:, d])
            nc.sync.dma_start(out=m8t[:], in_=m8_d[:])
            # Scalar (HWDGE #2): one m chunk, off the ring's critical path.
            nc.scalar.dma_start(out=m16[:, 4], in_=m_d[:][:, 4])
            # SWDGE ring, in need-order:
            for ch in range(2):  # x head strips: gate the first conv tile
                nc.gpsimd.dma_start(
                    out=xts[0][ch][:, 0:S0], in_=xp_d[:][0, ch, :, 0:S0]
                )
            nc.gpsimd.dma_start(out=bias_sb[:], in_=b_d[:])
            for d in (5, 6):
                nc.gpsimd.dma_start(out=m16[:, d], in_=m_d[:][:, d])
            for t in range(2):  # x8 strips: first DR is ~2.6us after conv start
                nc.gpsimd.dma_start(out=x8ts[0][t][:, 0:4], in_=x8_d[:][0, t, :, 0:4])
            # img0 remainder in progressive chunks: a tile's LDWEIGHTS waits
            # on the completion semaphore of the chunk containing its window
            for ch in range(2):
                nc.gpsimd.dma_start(
                    out=xts[0][ch][:, S0:2048], in_=xp_d[:][0, ch, :, S0:2048]
                )
            for t in range(2):
                nc.gpsimd.dma_start(out=x8ts[0][t][:, 4:NT], in_=x8_d[:][0, t, :, 4:NT])
            for ch in range(2):
                nc.gpsimd.dma_start(
                    out=xts[0][ch][:, 2048:XF], in_=xp_d[:][0, ch, :, 2048:XF]
                )
            for ch in range(2):  # img1 whole
                nc.gpsimd.dma_start(out=xts[1][ch][:], in_=xp_d[:][1, ch])
            for t in range(2):
                nc.gpsimd.dma_start(out=x8ts[1][t][:], in_=x8_d[:][1, t])

            # ---- PE pre-warm, ramped: 2 free-running matmuls as soon as the
            # engine boots, then 1 per early M-chunk arrival; the conv loop's
            # own DMA gates continue the ramp to 100% duty ----
            warm16 = const.tile([128, 256], F16, tag="warm16")
            nc.vector.memset(warm16[:], 0.0)
            wps = psum_pool.tile([128, 512], F32, tag="ps", name="warm")
            for _ in range(2):
                nc.tensor.matmul(
                    wps[:, 0:256], lhsT=warm16[:, 0:128], rhs=warm16[:]
                )
            for d in (0, 1, 2):
                for _ in range(2):
                    nc.tensor.matmul(
                        wps[:, 0:256], lhsT=warm16[:, 0:128], rhs=m16[:, d, 0, :]
                    )

            # ---- main conv loop ----
            # tile j = padded positions q in [66 + 128j, 66 + 128j + 128);
            # tap d reads the flat buffer at 67 + 128j + delta_d (contiguous).
            # pt accumulates taps d0-d6 (12 fp16 matmuls per half); pt8 is
            # preloaded with bias*s by the Scalar engine and takes taps d3/d7
            # as fp8 DoubleRow matmuls. The DVE cannot read two PSUM inputs
            # in one op (NCC_IBVF027), so the DR matmuls run FIRST in each
            # pair (except pair 0, whose startup gating wants the fp16 taps
            # first) and the Scalar engine rescales pt8 -> t8 (SBUF) while
            # the fp16 taps are still accumulating; the only post-matmul op
            # is then one DVE add + the store.
            deltas = [-(dy * HP + dx) for (dx, dy) in DIRECTIONS]
            pair_idx = 0
            for img in range(BPC):
                x0, x1 = xts[img][0], xts[img][1]
                for jp in range((NT + 1) // 2):
                    pair = [j for j in (2 * jp, 2 * jp + 1) if j < NT]
                    pw = 256 * len(pair)
                    pt = psum_pool.tile([128, 512], F32, tag="ps", name=f"ps{img}_{jp}")
                    pt8 = psum_pool.tile(
                        [128, 512], F32, tag="ps", name=f"ps8{img}_{jp}"
                    )

                    def emit_dr(half, j):
                        for ti in range(2):
                            nc.tensor.matmul(
                                pt8[:, half * 256 : (half + 1) * 256],
                                lhsT=x8ts[img][ti][:, j],
                                rhs=m8t[:, ti],
                                start=(ti == 0),
                                stop=(ti == 1),
                                perf_mode=mybir.MatmulPerfMode.DoubleRow,
                            )

                    def emit_f16(half, j):
                        for di in range(8):
                            if di in FP8_TAPS:
                                continue
                            s = 67 + 128 * j + deltas[di]
                            for ch, xt in enumerate((x0, x1)):
                                nc.tensor.matmul(
                                    pt[:, half * 256 : (half + 1) * 256],
                                    lhsT=xt[:, s : s + 128],
                                    rhs=m16[:, di, ch, :],
                                    start=(di == 0 and ch == 0),
                                    stop=(di == 6 and ch == 1),
                                )

                    if pair_idx < 3:
                        # startup: m8/x8 strips arrive after the m16 front
                        # half, so the fp16 taps lead while data streams in
                        for half, j in enumerate(pair):
                            emit_f16(half, j)
                        for half, j in enumerate(pair):
                            emit_dr(half, j)
                    else:
                        # steady state: DR taps lead so the Scalar/GpSimd
                        # rescale+bias chain runs under the fp16 taps and
                        # only one DVE add remains after the last matmul
                        for half, j in enumerate(pair):
                            emit_dr(half, j)
                        for half, j in enumerate(pair):
                            emit_f16(half, j)
                    # pt8 rescale + bias add run on the (otherwise idle)
                    # Scalar and GpSimd engines while the fp16 taps are
                    # still accumulating into pt: the only op left on the
                    # post-matmul critical path is one DVE add.
                    t8a = osb_pool.tile([128, 512], F32, tag="t8a", name=f"t8a{img}_{jp}")
                    t8b = osb_pool.tile([128, 512], F32, tag="t8b", name=f"t8b{img}_{jp}")
                    nc.scalar.mul(t8a[:, :pw], pt8[:, :pw], inv_s)
                    nc.gpsimd.tensor_add(t8b[:, :pw], t8a[:, :pw], bias_sb[:, :pw])
                    ot = osb_pool.tile([128, 512], F16, tag="osb", name=f"ot{img}_{jp}")
                    nc.vector.tensor_add(ot[:, :pw], pt[:, :pw], t8b[:, :pw])
                    # store: out rows = img*NQ + 128*j + p, contiguous per tile
                    base_row = img * NQ + 128 * pair[0]
                    dst = out_d[:][base_row : base_row + 128 * len(pair), :].rearrange(
                        "(j p) o -> p j o", p=128
                    )
                    nc.sync.dma_start(
                        out=dst,
                        in_=ot[:, :pw].rearrange("p (j o) -> p j o", o=256),
                    )
                    pair_idx += 1

    nc.finalize()  # Bacc: run reg-alloc + sync-wait splitting before serialization
    return nc


def _host_prep(grid_embedding, Wd, Wc, bc):
    g = np.asarray(grid_embedding, dtype=np.float32)
    gpad = np.zeros((B, C, XF), np.float16)
    gview = gpad[:, :, 1 : 1 + HP * HP].reshape(B, C, HP, HP)
    gview[:, :, 1 : H + 1, 1 : W + 1] = g.transpose(0, 3, 1, 2)
    xp = gpad.reshape(B, 2, 128, XF)
    # fold: M[d, c, o] = sum_e Wd[d, c, e] * Wc[o, d*C + e], fp64 accumulate
    wcr = np.asarray(Wc, np.float64).reshape(C, 8, C)  # [o, d, e]
    M = np.einsum("dce,ode->dco", np.asarray(Wd, np.float64), wcr)
    m = np.ascontiguousarray(
        M.reshape(8, 2, 128, C).transpose(2, 0, 1, 3).astype(np.float16)
    )  # [p=c%128, d, c_chunk, o]
    # fp8 tap: global power-of-2 scale into e4m3 range
    import ml_dtypes

    absmax = max(float(np.abs(M[d]).max()) for d in FP8_TAPS)
    s = 2.0 ** np.floor(np.log2(448.0 / max(absmax, 1e-30) / 2.0))
    bias = np.zeros((128, 520), np.float32)  # cols 0:512 bias pair, col 512 = 1/s
    bias[:, 0:512] = np.tile(np.asarray(bc, np.float32), 2)
    m8 = np.ascontiguousarray(
        np.stack(
            [(M[d] * s).reshape(2, 128, C).transpose(1, 0, 2) for d in FP8_TAPS],
            axis=1,
        )
    ).astype(ml_dtypes.float8_e4m3)  # [p=c%128, tap, ch, o]
    x8flat = xp.astype(ml_dtypes.float8_e4m3)  # [img, ch, p, flat]
    slices = []
    for d in FP8_TAPS:
        off = 67 - (DIRECTIONS[d][1] * HP + DIRECTIONS[d][0])
        slices.append(
            x8flat[:, :, :, off : off + NT * 128]
            .reshape(B, 2, 128, NT, 128)
            .transpose(0, 2, 3, 1, 4)
        )  # [img, p, j, ch, 128]
    x8 = np.ascontiguousarray(np.stack(slices, axis=1))  # [img, tap, p, j, ch, w]
    bias[:, 512] = 1.0 / s
    return xp, m, bias, x8, m8


_NC_CACHE = {}


def _unpad_out(outpad_flat):
    # [NQ*images, 256] f16 -> [images, H, W, C]: rows are (hp-1, wp) for padded
    # rows hp in 1..64 and all wp in 0..65; discard wp 0 and 65.
    n_img = outpad_flat.shape[0] // NQ
    o = outpad_flat.reshape(n_img, H, HP, C)
    return o[:, :, 1 : W + 1, :]


def kernel(grid_embedding, Wd, Wc, bc):
    global LAST_RESULTS
    xp, m, bias, x8, m8 = _host_prep(grid_embedding, Wd, Wc, bc)

    if "nc" not in _NC_CACHE:
        _NC_CACHE["nc"] = build_bass()
    nc = _NC_CACHE["nc"]

    in_maps = [
        {
            "xp": np.ascontiguousarray(xp[core * BPC : (core + 1) * BPC]),
            "x8": np.ascontiguousarray(x8[core * BPC : (core + 1) * BPC]),
            "m": m,
            "m8": m8,
            "bias": bias,
        }
        for core in range(N_CORES)
    ]
    res = run_bass_kernel_spmd(nc, in_maps, core_ids=list(range(N_CORES)))
    LAST_RESULTS = res
    out = np.concatenate([_unpad_out(r["out"]) for r in res.results], axis=0)
    return np.ascontiguousarray(out.astype(np.float32))


if __name__ == "__main__":
    rng = np.random.default_rng(0)
    inputs = {
        "grid_embedding": rng.standard_normal((B, H, W, C), dtype=np.float32),
        "Wd": (rng.standard_normal((8, C, C)) * 0.01).astype(np.float32),
        "Wc": (rng.standard_normal((C, 8 * C)) * 0.02).astype(np.float32),
        "bc": (rng.standard_normal(C) * 0.02).astype(np.float32),
    }
    out = kernel(**inputs)
    print("out", out.shape, out.dtype)
